# revision 2
# baseline (speedup 1.0000x reference)
"""Trainium2 Bass kernel for nn_GatedBlock (moe_routing).

Math (reference collapses): the (NB,BS,BS) reshape of weight maps block k to
rows [128k, 128k+128) of weight, so
    out[b, i] = g[b, i // 128] * (x @ W.T)[b, i] + bias[i]
with g = sigmoid(x @ gate_w + gate_b), bottom-8 of 16 gates zeroed per row.

Sharding: output-dim (i) split 8 ways -> 256 rows of W (= 2 gate blocks) per
core.  Per-core inputs:
  pre (128, KT, 48) f32r  [x.T | gate_w[:, perm]] k-tile-major (gate cols
                          permuted so this core's two blocks sit at 0,1)
  xb  (128, KT, 32) bf16  x.T k-tile-major (stationary for the main matmul)
  rhs (128, KT, 256) bf16 W_shard.T k-tile-major
  epi (32, 272) f32       [bias_shard bcast | gate_b[perm] bcast]

The gate linear runs in f32r off `pre` (bf16 would risk flipping the top-8
boundary: min gate margin 3.4e-4 vs ~3e-4 bf16 logit noise); the main matmul
runs in bf16 (measured 1.9e-3 rel err vs the 2e-2 gate), which halves the
dominant W DMA vs f32/f32r.  Top-8 mask via vector.max + match_replace; the
whole gate chain hides under the W DMA phase.
"""

import sys

for _p in ("/opt/trn_rl_repo", "/root/.axon_site/_ro/trn_rl_repo"):
    if _p not in sys.path:
        sys.path.append(_p)

import os as _os

import numpy as np

B = 32          # batch
D = 2048        # model dim
NB = 16         # gate blocks
BLK = D // NB   # 128 output rows per gate block
N_CORES = 8
NOUT = D // N_CORES       # 256 output cols per core
KT = D // 128             # 16 k-tiles
NPRE = B + NB             # 48 = xT cols + gate cols in the gate array

MODE = _os.environ.get("GATED_MODE", "bf16")     # "f32r" | "bf16"
# k-tiles per rhs DMA, summing to KT, issued round-robin over the two HWDGE
# queues.  Finer groups overlap compute earlier but pay ~750ns issue each.
GROUPS = [int(v) for v in
          _os.environ.get("GATED_GROUPS", "3,3,3,3,3,1").split(",")]
DMA_ENGS = _os.environ.get("GATED_ENGS", "sync,scalar").split(",")
SPLIT_OUT = _os.environ.get("GATED_SPLIT_OUT", "1") == "1"

_compiled = {}


def _build(mode):
    import concourse.bacc as bacc
    import concourse.tile as tile
    import concourse.mybir as mybir

    f32 = mybir.dt.float32
    f32r = mybir.dt.float32r
    bf16 = mybir.dt.bfloat16
    mm_dt = f32r if mode == "f32r" else bf16

    nc = bacc.Bacc("TRN2", target_bir_lowering=False, debug=False,
                   num_devices=N_CORES)

    pre_d = nc.dram_tensor("pre", [128, KT, NPRE], f32r, kind="ExternalInput")
    if mode == "bf16":
        xb_d = nc.dram_tensor("xb", [128, KT, B], bf16, kind="ExternalInput")
    rhs_d = nc.dram_tensor("rhs", [128, KT, NOUT], mm_dt, kind="ExternalInput")
    epi_d = nc.dram_tensor("epi", [B, NOUT + NB], f32, kind="ExternalInput")
    out_d = nc.dram_tensor("out", [B, NOUT], f32, kind="ExternalOutput")

    with tile.TileContext(nc) as tc:
        with (
            tc.tile_pool(name="sb", bufs=1) as sb,
            tc.tile_pool(name="ps", bufs=1, space="PSUM") as psp,
        ):
            pre = sb.tile([128, KT, NPRE], f32r, name="pre_sb", tag="pre_sb")
            if mode == "bf16":
                xb = sb.tile([128, KT, B], bf16, name="xb_sb", tag="xb_sb")
            rhs = sb.tile([128, KT, NOUT], mm_dt, name="rhs_sb", tag="rhs_sb")
            epi = sb.tile([B, NOUT + NB], f32, name="epi_sb", tag="epi_sb")
            graw = sb.tile([B, NB], f32, name="graw", tag="graw")
            g = sb.tile([B, NB], f32, name="g", tag="g")
            m8 = sb.tile([B, 8], f32, name="m8", tag="m8")
            rep = sb.tile([B, NB], f32, name="rep", tag="rep")
            gk = sb.tile([B, NB], f32, name="gk", tag="gk")
            outt = sb.tile([B, NOUT], f32, name="outt", tag="outt")
            ps_g = psp.tile([B, NB], f32, name="ps_g", tag="ps_g")
            ps_m = [psp.tile([B, BLK], f32, name=f"ps_m{h}", tag=f"ps_m{h}")
                    for h in range(NOUT // BLK)]

            engs = [getattr(nc, e) for e in DMA_ENGS]

            # early loads: pre (gate data) on queue 0; stationary + epilogue
            # data on queue 1 — both land well before the W stream drains
            engs[0].dma_start(pre[:], pre_d.ap())
            if mode == "bf16":
                engs[1 % len(engs)].dma_start(xb[:], xb_d.ap())
            engs[1 % len(engs)].dma_start(epi[:], epi_d.ap())

            # rhs groups, round-robin over DMA queues
            assert sum(GROUPS) == KT, GROUPS
            di = int(_os.environ.get("GATED_DI0", "1"))
            t0 = 0
            for gsz in GROUPS:
                engs[di % len(engs)].dma_start(
                    rhs[:, t0:t0 + gsz, :],
                    rhs_d.ap()[:, t0:t0 + gsz, :],
                )
                t0 += gsz
                di += 1

            # gate linear: 16 tiny f32r matmuls off the early pre load
            for t in range(KT):
                nc.tensor.matmul(
                    ps_g[:], pre[:, t, :B], pre[:, t, B:NPRE],
                    start=(t == 0), stop=(t == KT - 1),
                )

            # gate chain (hides under the rhs DMA phase)
            nc.vector.tensor_add(graw[:], ps_g[:], epi[:, NOUT:NOUT + NB])
            nc.scalar.activation(g[:], graw[:],
                                 mybir.ActivationFunctionType.Sigmoid)
            nc.vector.max(m8[:], g[:])
            nc.vector.match_replace(rep[:], m8[:], g[:], 0.0)
            nc.vector.tensor_sub(gk[:], g[:], rep[:])

            # main matmul accumulation: two independent column-half chains so
            # the first half's epilogue + store can start one matmul earlier.
            # Per k-tile, half B runs before half A so chain A's last matmul
            # is the overall second-to-last.
            stat = xb if mode == "bf16" else pre
            nh = NOUT // BLK
            i = [0] * nh
            for t in range(KT):
                for h in reversed(range(nh)):
                    nc.tensor.matmul(
                        ps_m[h][:], stat[:, t, :B],
                        rhs[:, t, h * BLK:(h + 1) * BLK],
                        start=(i[h] == 0), stop=(i[h] == KT - 1),
                    )
                    i[h] += 1

            # out = psum * g[block] + bias; each half's store issues as soon
            # as that half is ready
            for h in range(nh):
                sl = slice(h * BLK, (h + 1) * BLK)
                nc.vector.scalar_tensor_tensor(
                    outt[:, sl], ps_m[h][:], gk[:, h:h + 1], epi[:, sl],
                    mybir.AluOpType.mult, mybir.AluOpType.add,
                )
                if SPLIT_OUT:
                    engs[h % len(engs)].dma_start(out_d.ap()[:, sl], outt[:, sl])
            if not SPLIT_OUT:
                nc.sync.dma_start(out_d.ap(), outt[:])

    nc.compile()
    return nc


def _tile_major(a):
    """(D, n) -> (128, KT, n) k-tile-major contiguous."""
    n = a.shape[1]
    return np.ascontiguousarray(a.reshape(KT, 128, n).transpose(1, 0, 2))


def build_in_maps(x, gate_w, gate_b, weight, bias):
    import ml_dtypes

    x = np.asarray(x, dtype=np.float32)
    gate_w = np.asarray(gate_w, dtype=np.float32)
    gate_b = np.asarray(gate_b, dtype=np.float32)
    weight = np.asarray(weight, dtype=np.float32)
    bias = np.asarray(bias, dtype=np.float32)

    xT = np.ascontiguousarray(x.T)                               # (2048, 32)
    xb_tm = _tile_major(xT.astype(ml_dtypes.bfloat16))
    in_maps = []
    for c in range(N_CORES):
        perm = [2 * c, 2 * c + 1] + [k for k in range(NB)
                                     if k not in (2 * c, 2 * c + 1)]
        pre = np.concatenate([xT, gate_w[:, perm]], axis=1)      # (2048, 48)
        w_shard = np.ascontiguousarray(weight[c * NOUT:(c + 1) * NOUT, :].T)
        epi = np.concatenate([
            np.broadcast_to(bias[c * NOUT:(c + 1) * NOUT], (B, NOUT)),
            np.broadcast_to(gate_b[perm], (B, NB)),
        ], axis=1).astype(np.float32)
        m = {"pre": _tile_major(pre), "epi": np.ascontiguousarray(epi)}
        if MODE == "bf16":
            m["xb"] = xb_tm
            m["rhs"] = _tile_major(w_shard.astype(ml_dtypes.bfloat16))
        else:
            m["rhs"] = _tile_major(w_shard)
        in_maps.append(m)
    return in_maps


def _ensure_ntff_hook():
    """If a caller sets BASS_TRACE, run_bass_kernel_spmd imports
    antenv.axon_hooks, which is missing in this image; provide a working
    ctypes-backed stub so tracing degrades gracefully instead of raising."""
    try:
        from antenv.axon_hooks import get_axon_ntff_profile_hook  # noqa: F401
        return
    except ImportError:
        pass
    import contextlib
    import ctypes
    import types

    try:
        lib = ctypes.CDLL("/opt/axon/libaxon_pjrt.so")
        assert hasattr(lib, "axon_start_nrt_profile")
        lib.axon_start_nrt_profile.argtypes = [
            ctypes.POINTER(ctypes.c_int64), ctypes.c_size_t]
        lib.axon_start_nrt_profile.restype = ctypes.c_int64
        lib.axon_stop_nrt_profile.argtypes = [ctypes.c_char_p]
        lib.axon_stop_nrt_profile.restype = ctypes.c_int64

        @contextlib.contextmanager
        def _hook(output_dir, device_ids):
            import jax
            jax.devices()
            if device_ids:
                ids = (ctypes.c_int64 * len(device_ids))(*device_ids)
                rc = lib.axon_start_nrt_profile(ids, len(device_ids))
            else:
                rc = lib.axon_start_nrt_profile(None, 0)
            if rc != 0:
                raise RuntimeError(f"axon_start_nrt_profile rc={rc}")
            try:
                yield
            finally:
                lib.axon_stop_nrt_profile(str(output_dir).encode())

        hook = _hook
    except Exception:
        hook = None

    mod = types.ModuleType("antenv.axon_hooks")
    mod.get_axon_ntff_profile_hook = lambda: hook
    mod.set_axon_ntff_profile_hook = lambda h: None
    sys.modules["antenv.axon_hooks"] = mod


def kernel(x, gate_w, gate_b, weight, bias):
    _ensure_ntff_hook()
    from concourse.bass_utils import run_bass_kernel_spmd

    if MODE not in _compiled:
        _compiled[MODE] = _build(MODE)
    nc = _compiled[MODE]

    in_maps = build_in_maps(x, gate_w, gate_b, weight, bias)
    res = run_bass_kernel_spmd(nc, in_maps, list(range(N_CORES)))
    out = np.concatenate([res.results[c]["out"] for c in range(N_CORES)], axis=1)
    return out.astype(np.float32)
